# revision 1
# baseline (speedup 1.0000x reference)
"""Trainium2 Bass kernel for nn_Node3DEmbeddingv2 (gnn_message_passing).

Strategy (8 NeuronCores, SPMD, data-parallel over flattened (batch, query-row)):
  - 1536 query rows split into 8 x 192 (batch-aligned: 4 cores per batch).
  - Per core, per 32-row group: pairwise deltas vs all 768 keys on DVE,
    squared, reduced over xyz via a block-replicating matmul -> d^2 (PSUM),
    ACT sqrt -> d (replicated 3x across partitions).
  - d is split into 3 bf16 components (hi/mid/lo, exact to f32 precision);
    a [3,128]-ones bf16 matmul broadcasts each row's 768 distances across
    all 128 gaussian-channel partitions at full PE rate (f32 accumulate).
  - One ScalarE op per row computes the whole Gaussian:
      Derivative_Erf(scale_k * d + bias_k) = 2/sqrt(pi) * exp(-((d-m_k)/s_k)^2/2)
    with accum_out producing the sum over the 768 keys for free.
    (Fallback path: Square + Exp, two ACT passes, if the table is inaccurate.)
  - Channel constants 1/(sqrt(2 pi) s_k) (and the table constant) are applied
    post-reduction on the tiny [128, 192] summed tensor.
  - feature_proj MLP (gelu between two matmuls) on device; PE transposes the
    [E, rows] result back to row-major, adds the host-computed angle/time
    embedding tail, DMAs out [192, 512] per core.
  - Host (numpy, negligible): angle MLP, sinusoidal time embedding MLP,
    masking, per-core input prep; all heavy compute is on-device.
"""

import math

import numpy as np

# Problem constants (hardcoded per the task contract).
B, N, K, E = 2, 768, 128, 512
INTER = E // 2
NCORES = 8
RPC = (B * N) // NCORES  # 192 rows per core
GR = 32                  # rows per group
NGROUPS = RPC // GR      # 6 groups per core
PI_REF = 3.14159         # matches reference's gaussian constant

# Derivative_Erf table semantics: d/dx erf(x) = 2/sqrt(pi) * exp(-x^2).
# DERF_INV is the factor that converts the table output back to exp(-x^2).
DERF_INV = math.sqrt(math.pi) / 2.0

# Set to True to use the Square+Exp fallback instead of Derivative_Erf.
USE_FALLBACK_EXP = False
# Simulator/testing hook: replace Derivative_Erf by another func (e.g. Square).
_FUNC_OVERRIDE = None

_COMPILED = {}


def _enable_ldw_opt():
    """Flip walrus's redundant-LDWEIGHTS elimination on: our 384 broadcast
    matmuls reuse one stationary [3,128] ones matrix, and the per-matmul
    reload serializes ~134ns each on the PE. Correctness is re-verified
    end-to-end against the oracle after any compile-flag change."""
    from concourse import bass_utils

    if getattr(bass_utils, "_ldw_opt_patched", False):
        return
    orig = bass_utils.bir_verify_and_optimise

    def patched(*args, **kwargs):
        import subprocess

        orig_run = bass_utils.run_command

        def run_patched(argv, **kw):
            argv = [
                a.replace("--enable-ldw-opt=false", "--enable-ldw-opt=true")
                if isinstance(a, str) else a
                for a in argv
            ]
            return orig_run(argv, **kw)

        bass_utils.run_command = run_patched
        try:
            return orig(*args, **kwargs)
        finally:
            bass_utils.run_command = orig_run

    bass_utils.bir_verify_and_optimise = patched
    bass_utils._ldw_opt_patched = True


def _build_nc(use_fallback=None, func_override=None, gelu_override=None):
    import concourse.bass as bass
    import concourse.bacc as bacc
    from concourse import mybir
    from concourse.tile import TileContext

    # note: _enable_ldw_opt() breaks walrus codegen (standalone InstLdweights
    # with f32 matmuls in the module) — left available but unused

    if use_fallback is None:
        use_fallback = USE_FALLBACK_EXP
    f32 = mybir.dt.float32
    bf16 = mybir.dt.bfloat16
    AF = mybir.ActivationFunctionType

    nc = bacc.Bacc("TRN2", target_bir_lowering=False)

    # DRAM I/O (per-core values supplied via in_maps).
    posT = nc.dram_tensor("posT", [3, N], f32, kind="ExternalInput")
    qscal = nc.dram_tensor("qscal", [96, NGROUPS], f32, kind="ExternalInput")
    blk3 = nc.dram_tensor("blk3", [96, GR], f32, kind="ExternalInput")
    esc = nc.dram_tensor("esc", [K, 1], f32, kind="ExternalInput")
    ebi = nc.dram_tensor("ebi", [K, 1], f32, kind="ExternalInput")
    postc = nc.dram_tensor("postc", [K, 1], f32, kind="ExternalInput")
    w1 = nc.dram_tensor("w1", [K, K], f32, kind="ExternalInput")
    w2 = nc.dram_tensor("w2", [K, INTER], f32, kind="ExternalInput")
    ident = nc.dram_tensor("ident", [128, 128], f32, kind="ExternalInput")
    rest = nc.dram_tensor("rest", [RPC, E], f32, kind="ExternalInput")
    out = nc.dram_tensor("out", [RPC, E], f32, kind="ExternalOutput")

    with TileContext(nc) as tc:
        with tc.tile_pool(name="sb", bufs=1) as sb:
            # ---- constant loads ----
            pos_rep = sb.tile([96, N], f32, tag="pos_rep")
            nc.sync.dma_start(
                out=pos_rep,
                in_=bass.AP(tensor=posT, offset=0, ap=[[0, 32], [N, 3], [1, N]]),
            )
            q_sb = sb.tile([96, NGROUPS], f32, tag="q_sb")
            nc.sync.dma_start(out=q_sb, in_=qscal[:, :])
            blk_sb = sb.tile([96, GR], f32, tag="blk_sb")
            nc.sync.dma_start(out=blk_sb, in_=blk3[:, :])
            esc_sb = sb.tile([K, 1], f32, tag="esc_sb")
            nc.sync.dma_start(out=esc_sb, in_=esc[:, :])
            ebi_sb = sb.tile([K, 1], f32, tag="ebi_sb")
            nc.sync.dma_start(out=ebi_sb, in_=ebi[:, :])
            postc_sb = sb.tile([K, 1], f32, tag="postc_sb")
            nc.sync.dma_start(out=postc_sb, in_=postc[:, :])
            w1_sb = sb.tile([K, K], f32, tag="w1_sb")
            nc.sync.dma_start(out=w1_sb, in_=w1[:, :])
            w2_sb = sb.tile([K, INTER], f32, tag="w2_sb")
            nc.sync.dma_start(out=w2_sb, in_=w2[:, :])
            id_sb = sb.tile([128, 128], f32, tag="id_sb")
            nc.sync.dma_start(out=id_sb, in_=ident[:, :])
            ones3 = sb.tile([3, 128], bf16, tag="ones3")
            nc.vector.memset(ones3, 1.0)

            S = sb.tile([K, RPC], f32, tag="S")

            # Collapse the many input-DMA queue semaphores into one point so
            # downstream consumers never need more waits than the instruction
            # encoding allows.
            tc.strict_bb_all_engine_barrier()

            # ---- phase A: distances + bf16 splits for all 6 groups ----
            split_tiles = []
            with tc.tile_pool(name="psA", bufs=1, space="PSUM") as psA:
                for g in range(NGROUPS):
                    delta = sb.tile([96, N], f32, tag="delta", bufs=2)
                    nc.vector.tensor_scalar(
                        out=delta,
                        in0=pos_rep,
                        scalar1=q_sb[:, g : g + 1],
                        scalar2=None,
                        op0=mybir.AluOpType.subtract,
                    )
                    nc.vector.tensor_mul(delta, delta, delta)
                    psum_d2 = psA.tile([GR, N], f32, tag="d2", bufs=2)
                    nc.tensor.matmul(
                        psum_d2[:, 0:512], blk_sb, delta[:, 0:512],
                        start=True, stop=True,
                    )
                    nc.tensor.matmul(
                        psum_d2[:, 512:N], blk_sb, delta[:, 512:N],
                        start=True, stop=True,
                    )
                    d_sb = sb.tile([GR, N], f32, tag=f"d{g}")
                    nc.scalar.sqrt(d_sb, psum_d2)
                    # exact 3-way bf16 split: hi + mid + lo == d (f32 precision)
                    dh = sb.tile([GR, N], bf16, tag=f"dh{g}")
                    nc.vector.tensor_copy(dh, d_sb)
                    r1 = sb.tile([GR, N], f32, tag="r1", bufs=2)
                    nc.vector.tensor_sub(r1, d_sb, dh)
                    dm = sb.tile([GR, N], bf16, tag=f"dm{g}")
                    nc.vector.tensor_copy(dm, r1)
                    r2 = sb.tile([GR, N], f32, tag="r2", bufs=2)
                    nc.vector.tensor_sub(r2, r1, dm)
                    dl = sb.tile([GR, N], bf16, tag=f"dl{g}")
                    nc.vector.tensor_copy(dl, r2)
                    split_tiles.append((dh, dm, dl))

            # ---- phase B: broadcast + gaussian + key-sum per row ----
            derf_func = AF.Derivative_Erf
            if func_override is not None:
                derf_func = func_override
            with tc.tile_pool(name="psB", bufs=1, space="PSUM") as psB:
                ones2q = sb.tile([67, 128], bf16, tag="ones2q")
                nc.vector.memset(ones2q, 1.0)
                SG = GR // 2
                for g2 in range(NGROUPS * 2):
                    g, s = divmod(g2, 2)
                    # flatten 16 rows onto two PE quadrant trios (partitions
                    # 0..2 and 64..66): 8 rows each. Alternating matmul issue
                    # between the quadrants lets the PE pipeline them
                    # concurrently (~2x effective rate).
                    M_flat = sb.tile([67, SG * N // 2], bf16, tag="mflat", bufs=2)
                    for c, comp in enumerate(split_tiles[g]):
                        for q in range(2):
                            nc.gpsimd.dma_start(
                                out=M_flat[
                                    64 * q + c : 64 * q + c + 1, :
                                ].rearrange("p (a j) -> p a j", a=SG // 2),
                                in_=comp[
                                    SG * s + 8 * q : SG * s + 8 * (q + 1), :
                                ],
                            )
                    for a0 in range(0, SG, 4):
                        a = SG * s + a0
                        r = g * GR + a
                        unit = r // 4
                        # every 8th 4-row unit sums on the ScalarE accumulator
                        # (1-row activations with accum_out) to offload the DVE
                        act_accum_unit = (not use_fallback) and unit % 8 == 7
                        # 4-row macro unit: two 2-row PSUM tiles -> one 4-row
                        # gsc tile. Rows a0, a0+1 come from quadrant 0
                        # (partitions 0..2), rows a0+2, a0+3 (= slot a0, a0+1
                        # of the upper half) from quadrant 2 (partitions
                        # 64..66); issue alternates between the two so their
                        # matmuls overlap in the PE array.
                        gsc = sb.tile([K, 4, N], f32, tag="gsc", bufs=4)
                        mms = []
                        psums = []
                        base = (a0 // 4) * 2 * N  # slot pair 2u, 2u+1
                        for h in range(2):
                            psum_db = psB.tile([K, 2, N], f32, tag="db", bufs=2)
                            flat = psum_db.rearrange("k a j -> k (a j)")
                            qb = 64 * h
                            for lo in (0, 512, 1024):
                                mms.append(
                                    (
                                        flat[:, lo : lo + 512],
                                        ones2q[qb : qb + 3, :],
                                        M_flat[qb : qb + 3, base + lo : base + lo + 512],
                                        (qb, 0),
                                    )
                                )
                            psums.append(psum_db)
                        for idx in (0, 3, 1, 4, 2, 5):
                            out_ap, lhsT, rhs, tp = mms[idx]
                            nc.tensor.matmul(
                                out_ap, lhsT, rhs,
                                start=True, stop=True, tile_position=tp,
                            )
                        for h in range(2):
                            psum_db = psums[h]
                            if act_accum_unit:
                                for q in range(2):
                                    nc.scalar.activation(
                                        out=gsc[:, 2 * h + q, :],
                                        in_=psum_db[:, q, :],
                                        func=derf_func,
                                        bias=ebi_sb,
                                        scale=esc_sb,
                                        accum_out=S[:, r + 2 * h + q : r + 2 * h + q + 1],
                                    )
                            elif not use_fallback:
                                nc.scalar.activation(
                                    out=gsc[:, 2 * h : 2 * h + 2, :],
                                    in_=psum_db,
                                    func=derf_func,
                                    bias=ebi_sb,
                                    scale=esc_sb,
                                )
                            else:
                                zsq = sb.tile([K, 2, N], f32, tag="zsq", bufs=3)
                                nc.scalar.activation(
                                    out=zsq, in_=psum_db,
                                    func=AF.Square, bias=ebi_sb, scale=esc_sb,
                                )
                                nc.scalar.activation(
                                    out=gsc[:, 2 * h : 2 * h + 2, :], in_=zsq,
                                    func=AF.Exp, bias=postc_sb, scale=-0.5,
                                )
                        if not act_accum_unit:
                            # key-axis sum on DVE (4 rows per op)
                            nc.vector.reduce_sum(
                                out=S[:, r : r + 4], in_=gsc,
                                axis=mybir.AxisListType.X,
                            )

            # ---- phase C: channel constants + feature_proj MLP + output ----
            # processed in two 96-row chunks so the second half of phase B can
            # still be running while the first chunk's MLP drains
            with tc.tile_pool(name="psC", bufs=1, space="PSUM") as psC:
                gelu_func = AF.Gelu if gelu_override is None else gelu_override
                for t in range(2):
                    rows = slice(96 * t, 96 * (t + 1))
                    if not use_fallback:
                        nc.vector.tensor_scalar_mul(
                            S[:, rows], S[:, rows], postc_sb
                        )
                    psum_h = psC.tile([K, 96], f32, tag="mlp", bufs=2)
                    nc.tensor.matmul(psum_h, w1_sb, S[:, rows], start=True, stop=True)
                    h_sb = sb.tile([K, 96], f32, tag="h_sb", bufs=2)
                    nc.scalar.activation(h_sb, psum_h, gelu_func)
                    o_sb = sb.tile([128, 2, 96], f32, tag="o_sb", bufs=2)
                    for e in range(2):
                        psum_o = psC.tile([128, 96], f32, tag="mlp", bufs=2)
                        nc.tensor.matmul(
                            psum_o, w2_sb[:, 128 * e : 128 * (e + 1)], h_sb,
                            start=True, stop=True,
                        )
                        nc.vector.tensor_copy(o_sb[:, e, :], psum_o)
                    out_sb = sb.tile([96, E], f32, tag=f"out{t}")
                    nc.gpsimd.dma_start(
                        out=out_sb, in_=rest[96 * t : 96 * (t + 1), :]
                    )
                    for e in range(2):
                        psum_t = psC.tile([96, 128], f32, tag="tr", bufs=2)
                        nc.tensor.transpose(psum_t, o_sb[:, e, :], id_sb)
                        nc.vector.tensor_add(
                            out_sb[:, 128 * e : 128 * (e + 1)],
                            out_sb[:, 128 * e : 128 * (e + 1)],
                            psum_t,
                        )
                    nc.sync.dma_start(
                        out=out[96 * t : 96 * (t + 1), :], in_=out_sb
                    )

    nc.compile()
    return nc


# ---------------- host-side reference tails (numpy, f32) ----------------

def _erf_np(x):
    try:
        from scipy.special import erf
        return erf(x).astype(np.float32)
    except ImportError:
        f = np.frompyfunc(math.erf, 1, 1)
        return f(x.astype(np.float64)).astype(np.float32)


def _gelu_np(x):
    x = x.astype(np.float32)
    return (x * 0.5 * (1.0 + _erf_np(x / np.float32(math.sqrt(2.0))))).astype(
        np.float32
    )


def _silu_np(x):
    x = x.astype(np.float32)
    return (x / (1.0 + np.exp(-x))).astype(np.float32)


def _timestep_emb_np(t, dim):
    half = dim // 2
    freqs = np.exp(
        -np.log(10000.0) * np.arange(half, dtype=np.float32) / np.float32(half)
    ).astype(np.float32)
    a = t.astype(np.float32)[:, None] * freqs[None, :]
    return np.concatenate([np.sin(a), np.cos(a)], axis=-1).astype(np.float32)


def _host_tails(angle, mask_pos, time_pos, ang_w1, ang_w2, t_w1, t_b1, t_w2, t_b2):
    """rest[b, n, :] with rest[..., :INTER] = time_emb[..., :INTER] and
    rest[..., INTER:] = ang_f + time_emb[..., INTER:]."""
    angle = np.asarray(angle, np.float32)
    ang = np.where(np.isposinf(angle), np.float32(0.0), angle).astype(np.float32)
    ang_f = _gelu_np(ang @ np.asarray(ang_w1, np.float32)) @ np.asarray(
        ang_w2, np.float32
    )  # [B, N, INTER]

    def time_mlp(t):
        e = _timestep_emb_np(t, E)
        h = _silu_np(e @ np.asarray(t_w1, np.float32) + np.asarray(t_b1, np.float32))
        return (h @ np.asarray(t_w2, np.float32) + np.asarray(t_b2, np.float32)).astype(
            np.float32
        )

    tp = np.asarray(time_pos)
    te = time_mlp(tp)[:, None, :]                 # [B, 1, E]
    t0e = time_mlp(np.zeros_like(tp))[:, None, :]
    mask = np.asarray(mask_pos, bool)             # [B, N, 1]
    time_emb = np.where(mask, te, t0e).astype(np.float32)  # [B, N, E]

    rest = time_emb.copy()
    rest[..., INTER:] += ang_f.astype(np.float32)
    return rest.astype(np.float32)


def _prep_in_maps(pos, angle, padding_mask, mask_pos, time_pos,
                  means, stds, fp_w1, fp_w2, ang_w1, ang_w2,
                  t_w1, t_b1, t_w2, t_b2, use_fallback=None):
    if use_fallback is None:
        use_fallback = USE_FALLBACK_EXP
    pos = np.asarray(pos, np.float32)
    pad = np.asarray(padding_mask, bool)

    s = (np.abs(np.asarray(stds, np.float32)) + np.float32(0.01)).astype(np.float32)
    m = np.asarray(means, np.float32)
    inv_s = (np.float32(1.0) / s).astype(np.float32)
    if not use_fallback:
        # Derivative_Erf(x) with x = (d - m)/(s*sqrt(2))
        esc_v = (inv_s / np.float32(math.sqrt(2.0))).astype(np.float32)
        ebi_v = (-m * esc_v).astype(np.float32)
        postc_v = (
            np.float32(DERF_INV) / (np.float32(math.sqrt(2.0 * PI_REF)) * s)
        ).astype(np.float32)
    else:
        # Square then Exp(-0.5 z^2 + log c)
        esc_v = inv_s.astype(np.float32)
        ebi_v = (-m * inv_s).astype(np.float32)
        postc_v = np.log(
            np.float32(1.0) / (np.float32(math.sqrt(2.0 * PI_REF)) * s)
        ).astype(np.float32)

    blk3 = np.zeros((96, GR), np.float32)
    for p in range(96):
        blk3[p, p // 3] = 1.0

    rest = _host_tails(
        angle, mask_pos, time_pos, ang_w1, ang_w2, t_w1, t_b1, t_w2, t_b2
    )

    ident = np.eye(128, dtype=np.float32)
    w1_v = np.asarray(fp_w1, np.float32)
    w2_v = np.asarray(fp_w2, np.float32)

    in_maps = []
    for c in range(NCORES):
        b = c // (NCORES // B)
        r0 = (c % (NCORES // B)) * RPC
        posT = pos[b].T.copy()  # [3, N]
        if pad[b].any():
            posT[:, pad[b]] = np.float32(1.0e6)
        # phase-A partition rr holds the query row that lands on PE quadrant
        # 0 (first 8 of each 16-row subgroup) or quadrant 2 (last 8), so the
        # M_flat flatten DMAs stay partition-contiguous while consecutive
        # device rows alternate quadrants (rows a0,a0+1 -> Q0; a0+2,a0+3 -> Q2)
        perm16 = np.array([0, 1, 4, 5, 8, 9, 12, 13, 2, 3, 6, 7, 10, 11, 14, 15])
        perm = np.concatenate([perm16, 16 + perm16])
        qscal = np.empty((96, NGROUPS), np.float32)
        for g in range(NGROUPS):
            rows = pos[b, r0 + g * GR : r0 + (g + 1) * GR, :][perm]  # [32, 3]
            qscal[:, g] = rows.reshape(-1)
        in_maps.append(
            {
                "posT": np.ascontiguousarray(posT, np.float32),
                "qscal": qscal,
                "blk3": blk3,
                "esc": esc_v.reshape(K, 1),
                "ebi": ebi_v.reshape(K, 1),
                "postc": postc_v.reshape(K, 1),
                "w1": w1_v,
                "w2": w2_v,
                "ident": ident,
                "rest": np.ascontiguousarray(rest[b, r0 : r0 + RPC, :], np.float32),
            }
        )
    return in_maps


def kernel(pos, angle, node_type_edge, padding_mask, mask_aa, mask_pos, time_pos,
           means, stds, fp_w1, fp_w2, ang_w1, ang_w2, t_w1, t_b1, t_w2, t_b2):
    from concourse.bass_utils import run_bass_kernel_spmd

    key = ("nc", USE_FALLBACK_EXP, _FUNC_OVERRIDE)
    if key not in _COMPILED:
        _COMPILED[key] = _build_nc(func_override=_FUNC_OVERRIDE)
    nc = _COMPILED[key]

    in_maps = _prep_in_maps(
        pos, angle, padding_mask, mask_pos, time_pos, means, stds,
        fp_w1, fp_w2, ang_w1, ang_w2, t_w1, t_b1, t_w2, t_b2,
    )
    res = run_bass_kernel_spmd(nc, in_maps, core_ids=list(range(NCORES)))
    outs = [np.asarray(res.results[c]["out"], np.float32) for c in range(NCORES)]
    full = np.concatenate(outs, axis=0).reshape(B, N, E)
    return full



# revision 3
# speedup vs baseline: 1.0524x; 1.0524x over previous
"""Trainium2 Bass kernel for nn_Node3DEmbeddingv2 (gnn_message_passing).

Strategy (8 NeuronCores, SPMD, data-parallel over flattened (batch, query-row)):
  - 1536 query rows split into 8 x 192 (4 cores per batch).
  - Phase A (per 96-row chunk): pairwise deltas vs all 768 keys on DVE,
    squared, reduced over xyz via a block-ones f32 matmul into a shared
    [96,768] PSUM (tile_position stacks 3 groups of 32 rows), one ACT sqrt,
    then an exact 3-way fp16 split of d (33 mantissa bits > f32's 24).
  - Phase B (per 24-row block): flatten the fp16 d-components into 4
    PE-quadrant streams ([99,4608] moving tile, trios at partitions
    0/32/64/96), broadcast each row's 768 distances across all 128
    gaussian-channel partitions with [3,128]-ones fp16 matmuls
    (tile_position packs 4 concurrent 32-row PE tiles; issue alternates
    quadrants so matmuls overlap in the array). One ScalarE op per
    [128,1536] PSUM unit computes the whole Gaussian:
      Derivative_Erf(scale_k * d + bias_k) = 2/sqrt(pi) * exp(-((d-m_k)/s_k)^2/2)
    writing fp16. The key-axis sum runs as an in-place halving add-tree on
    DVE (fp16 tensor_tensor = 2x perf mode; tensor_reduce is 1x-capped) down
    to width 6, then one tiny f32 tensor_reduce into S.
  - Phase C: channel constants on the summed [128,192] tensor, f32
    feature_proj MLP (gelu between two matmuls), DMA-transpose (fp16) back
    to row-major, add the host-computed angle/time tail, DMA out [192,512].
  - Host (numpy, negligible): angle MLP, sinusoidal time embedding MLP,
    masking, per-core input prep; all heavy compute is on-device.
"""

import math

import numpy as np

# Problem constants (hardcoded per the task contract).
B, N, K, E = 2, 768, 128, 512
INTER = E // 2
NCORES = 8
RPC = (B * N) // NCORES  # 192 rows per core
PI_REF = 3.14159         # matches reference's gaussian constant

NCHUNK = 2               # 96-row phase-A chunks per core
NBLOCK = 8               # 24-row phase-B blocks per core
BR = 24                  # rows per block
QR = 6                   # rows per PE quadrant stream per block

# Derivative_Erf table semantics: d/dx erf(x) = 2/sqrt(pi) * exp(-x^2).
# DERF_INV converts the table output back to exp(-x^2).
DERF_INV = math.sqrt(math.pi) / 2.0

_COMPILED = {}


def _build_nc():
    import concourse.bass as bass
    import concourse.bacc as bacc
    from concourse import mybir
    from concourse.tile import TileContext

    f32 = mybir.dt.float32
    f16 = mybir.dt.float16
    AF = mybir.ActivationFunctionType

    nc = bacc.Bacc("TRN2", target_bir_lowering=False)

    posT = nc.dram_tensor("posT", [3, N], f32, kind="ExternalInput")
    qscal = nc.dram_tensor("qscal", [96, 6], f32, kind="ExternalInput")
    blk3 = nc.dram_tensor("blk3", [96, 32], f32, kind="ExternalInput")
    esc = nc.dram_tensor("esc", [K, 1], f32, kind="ExternalInput")
    ebi = nc.dram_tensor("ebi", [K, 1], f32, kind="ExternalInput")
    postc = nc.dram_tensor("postc", [K, 1], f32, kind="ExternalInput")
    w1 = nc.dram_tensor("w1", [K, K], f32, kind="ExternalInput")
    w2 = nc.dram_tensor("w2", [K, INTER], f32, kind="ExternalInput")
    rest = nc.dram_tensor("rest", [RPC, E], f32, kind="ExternalInput")
    out = nc.dram_tensor("out", [RPC, E], f32, kind="ExternalOutput")

    with TileContext(nc) as tc:
        with nc.allow_low_precision(reason="fp16 gaussian accumulate, verified vs oracle"), \
             tc.tile_pool(name="sb", bufs=1) as sb:
            # ---- constant loads ----
            pos_rep = sb.tile([96, N], f32, tag="pos_rep")
            nc.sync.dma_start(
                out=pos_rep,
                in_=bass.AP(tensor=posT, offset=0, ap=[[0, 32], [N, 3], [1, N]]),
            )
            q_sb = sb.tile([96, 6], f32, tag="q_sb")
            nc.sync.dma_start(out=q_sb, in_=qscal[:, :])
            blk_sb = sb.tile([96, 32], f32, tag="blk_sb")
            nc.sync.dma_start(out=blk_sb, in_=blk3[:, :])
            esc_sb = sb.tile([K, 1], f32, tag="esc_sb")
            nc.sync.dma_start(out=esc_sb, in_=esc[:, :])
            ebi_sb = sb.tile([K, 1], f32, tag="ebi_sb")
            nc.sync.dma_start(out=ebi_sb, in_=ebi[:, :])
            postc_sb = sb.tile([K, 1], f32, tag="postc_sb")
            nc.sync.dma_start(out=postc_sb, in_=postc[:, :])
            w1_sb = sb.tile([K, K], f32, tag="w1_sb")
            nc.sync.dma_start(out=w1_sb, in_=w1[:, :])
            w2_sb = sb.tile([K, INTER], f32, tag="w2_sb")
            nc.sync.dma_start(out=w2_sb, in_=w2[:, :])
            ones99 = sb.tile([99, 128], f16, tag="ones99")
            nc.vector.memset(ones99, 1.0)

            S = sb.tile([K, RPC], f32, tag="S")

            # Collapse the input-DMA queue semaphores into one point.
            tc.strict_bb_all_engine_barrier()

            dcomps = {}

            def phase_a(c):
                # distances + fp16 splits for one 96-row chunk
                d2p = psA.tile([96, N], f32, tag="d2p", bufs=1)
                for g in range(3):
                    gg = 3 * c + g
                    delta = sb.tile([96, N], f32, tag="delta", bufs=2)
                    nc.vector.tensor_scalar(
                        out=delta,
                        in0=pos_rep,
                        scalar1=q_sb[:, gg : gg + 1],
                        scalar2=None,
                        op0=mybir.AluOpType.subtract,
                    )
                    nc.vector.tensor_mul(delta, delta, delta)
                    nc.tensor.matmul(
                        d2p[32 * g : 32 * g + 32, 0:512],
                        blk_sb, delta[:, 0:512],
                        start=True, stop=True, tile_position=(0, 32 * g),
                    )
                    nc.tensor.matmul(
                        d2p[32 * g : 32 * g + 32, 512:N],
                        blk_sb, delta[:, 512:N],
                        start=True, stop=True, tile_position=(0, 32 * g),
                    )
                d32 = sb.tile([96, N], f32, tag="d32", bufs=2)
                nc.scalar.sqrt(d32, d2p)
                dc = sb.tile([96, 3, N], f16, tag="dcomp", bufs=2)
                nc.vector.tensor_copy(dc[:, 0, :], d32)
                r1 = sb.tile([96, N], f32, tag="r1", bufs=2)
                nc.vector.tensor_sub(r1, d32, dc[:, 0, :])
                nc.vector.tensor_copy(dc[:, 1, :], r1)
                r2 = sb.tile([96, N], f32, tag="r2", bufs=2)
                nc.vector.tensor_sub(r2, r1, dc[:, 1, :])
                nc.vector.tensor_copy(dc[:, 2, :], r2)
                dcomps[c] = dc

            def phase_b(b):
                # broadcast + gaussian + key-sum for one 24-row block
                dc = dcomps[b // 4]
                p0 = BR * (b % 4)
                mfl = sb.tile([99, QR * N], f16, tag="mflat", bufs=2)
                for comp in range(3):
                    nc.gpsimd.dma_start(
                        out=mfl[comp : 99 : 32, :].rearrange(
                            "p (i j) -> p i j", i=QR
                        ),
                        in_=dc[p0 : p0 + BR, comp, :],
                    )
                gsc = sb.tile([K, 4, QR, N], f16, tag="gsc", bufs=2)
                for k3 in range(3):
                    for qp in range(2):  # quadrant pairs (0,1) and (2,3)
                        pus = []
                        for q in (2 * qp, 2 * qp + 1):
                            pu = psB.tile([K, 1536], f32, tag="pu", bufs=2)
                            pus.append((q, pu))
                        for w in range(3):  # alternate quadrants per matmul
                            for q, pu in pus:
                                lo = 1536 * k3 + 512 * w
                                nc.tensor.matmul(
                                    pu[:, 512 * w : 512 * w + 512],
                                    ones99[32 * q : 32 * q + 3, :],
                                    mfl[32 * q : 32 * q + 3, lo : lo + 512],
                                    start=True, stop=True,
                                    tile_position=(32 * q, 0),
                                )
                        for q, pu in pus:
                            nc.scalar.activation(
                                out=gsc[:, q, 2 * k3 : 2 * k3 + 2, :],
                                in_=pu,
                                func=AF.Derivative_Erf,
                                bias=ebi_sb,
                                scale=esc_sb,
                            )
                # in-place fp16 halving add-tree over the key axis
                t = gsc.rearrange("k q i j -> k (q i) j")
                wdt = N // 2
                while wdt >= 6:
                    nc.vector.tensor_add(
                        t[:, :, 0:wdt], t[:, :, 0:wdt], t[:, :, wdt : 2 * wdt]
                    )
                    wdt //= 2
                nc.vector.tensor_reduce(
                    out=S[:, BR * b : BR * (b + 1)],
                    in_=t[:, :, 0:6],
                    axis=mybir.AxisListType.X,
                    op=mybir.AluOpType.add,
                )

            with tc.tile_pool(name="psA", bufs=1, space="PSUM") as psA, \
                 tc.tile_pool(name="psB", bufs=1, space="PSUM") as psB:
                phase_a(0)
                for b in range(NBLOCK):
                    if b == 2:
                        phase_a(1)
                    phase_b(b)

            # ---- phase C: channel constants + feature_proj MLP + output ----
            with tc.tile_pool(name="psC", bufs=1, space="PSUM") as psC:
                for st in range(2):
                    rows = slice(96 * st, 96 * (st + 1))
                    out_sb = sb.tile([96, E], f32, tag="out_sb", bufs=2)
                    nc.sync.dma_start(out=out_sb, in_=rest[rows, :])
                    nc.vector.tensor_scalar_mul(S[:, rows], S[:, rows], postc_sb)
                    psum_h = psC.tile([K, 96], f32, tag="mlp_h", bufs=2)
                    nc.tensor.matmul(psum_h, w1_sb, S[:, rows], start=True, stop=True)
                    h_sb = sb.tile([K, 96], f32, tag="h_sb", bufs=2)
                    nc.scalar.activation(h_sb, psum_h, AF.Gelu)
                    o16 = sb.tile([K, 2, 128], f16, tag="o16", bufs=2)
                    nc.vector.memset(o16, 0.0)
                    for e in range(2):
                        psum_o = psC.tile([K, 96], f32, tag="mlp_o", bufs=2)
                        nc.tensor.matmul(
                            psum_o, w2_sb[:, 128 * e : 128 * (e + 1)], h_sb,
                            start=True, stop=True,
                        )
                        nc.scalar.copy(o16[:, e, 0:96], psum_o)
                    tr16 = sb.tile([128, 2 * K], f16, tag="tr16", bufs=2)
                    for e in range(2):
                        nc.sync.dma_start_transpose(
                            tr16[:, 128 * e : 128 * (e + 1)], o16[:, e, :]
                        )
                    nc.vector.tensor_add(
                        out_sb[:, 0:INTER], out_sb[:, 0:INTER], tr16[0:96, :]
                    )
                    nc.sync.dma_start(out=out[rows, :], in_=out_sb)

    nc.compile()
    return nc


# ---------------- host-side reference tails (numpy, f32) ----------------

def _erf_np(x):
    try:
        from scipy.special import erf
        return erf(x).astype(np.float32)
    except ImportError:
        f = np.frompyfunc(math.erf, 1, 1)
        return f(x.astype(np.float64)).astype(np.float32)


def _gelu_np(x):
    x = x.astype(np.float32)
    return (x * 0.5 * (1.0 + _erf_np(x / np.float32(math.sqrt(2.0))))).astype(
        np.float32
    )


def _silu_np(x):
    x = x.astype(np.float32)
    return (x / (1.0 + np.exp(-x))).astype(np.float32)


def _timestep_emb_np(t, dim):
    half = dim // 2
    freqs = np.exp(
        -np.log(10000.0) * np.arange(half, dtype=np.float32) / np.float32(half)
    ).astype(np.float32)
    a = t.astype(np.float32)[:, None] * freqs[None, :]
    return np.concatenate([np.sin(a), np.cos(a)], axis=-1).astype(np.float32)


def _host_tails(angle, mask_pos, time_pos, ang_w1, ang_w2, t_w1, t_b1, t_w2, t_b2):
    """rest[b, n, :] with rest[..., :INTER] = time_emb[..., :INTER] and
    rest[..., INTER:] = ang_f + time_emb[..., INTER:]."""
    angle = np.asarray(angle, np.float32)
    ang = np.where(np.isposinf(angle), np.float32(0.0), angle).astype(np.float32)
    ang_f = _gelu_np(ang @ np.asarray(ang_w1, np.float32)) @ np.asarray(
        ang_w2, np.float32
    )  # [B, N, INTER]

    def time_mlp(t):
        e = _timestep_emb_np(t, E)
        h = _silu_np(e @ np.asarray(t_w1, np.float32) + np.asarray(t_b1, np.float32))
        return (h @ np.asarray(t_w2, np.float32) + np.asarray(t_b2, np.float32)).astype(
            np.float32
        )

    tp = np.asarray(time_pos)
    te = time_mlp(tp)[:, None, :]                 # [B, 1, E]
    t0e = time_mlp(np.zeros_like(tp))[:, None, :]
    mask = np.asarray(mask_pos, bool)             # [B, N, 1]
    time_emb = np.where(mask, te, t0e).astype(np.float32)  # [B, N, E]

    rest = time_emb.copy()
    rest[..., INTER:] += ang_f.astype(np.float32)
    return rest.astype(np.float32)


def _prep_in_maps(pos, angle, padding_mask, mask_pos, time_pos,
                  means, stds, fp_w1, fp_w2, ang_w1, ang_w2,
                  t_w1, t_b1, t_w2, t_b2):
    pos = np.asarray(pos, np.float32)
    pad = np.asarray(padding_mask, bool)

    s = (np.abs(np.asarray(stds, np.float32)) + np.float32(0.01)).astype(np.float32)
    m = np.asarray(means, np.float32)
    inv_s = (np.float32(1.0) / s).astype(np.float32)
    # Derivative_Erf(x) with x = (d - m)/(s*sqrt(2))
    esc_v = (inv_s / np.float32(math.sqrt(2.0))).astype(np.float32)
    ebi_v = (-m * esc_v).astype(np.float32)
    postc_v = (
        np.float32(DERF_INV) / (np.float32(math.sqrt(2.0 * PI_REF)) * s)
    ).astype(np.float32)

    blk3 = np.zeros((96, 32), np.float32)
    for p in range(96):
        blk3[p, p // 3] = 1.0

    rest = _host_tails(
        angle, mask_pos, time_pos, ang_w1, ang_w2, t_w1, t_b1, t_w2, t_b2
    )

    w1_v = np.asarray(fp_w1, np.float32)
    w2_v = np.asarray(fp_w2, np.float32)

    in_maps = []
    for c in range(NCORES):
        b = c // (NCORES // B)
        r0 = (c % (NCORES // B)) * RPC
        posT = pos[b].T.copy()  # [3, N]
        if pad[b].any():
            posT[:, pad[b]] = np.float32(1.0e6)
        qscal = np.empty((96, 6), np.float32)
        for g in range(6):
            rows = pos[b, r0 + g * 32 : r0 + (g + 1) * 32, :]  # [32, 3]
            qscal[:, g] = rows.reshape(-1)
        in_maps.append(
            {
                "posT": np.ascontiguousarray(posT, np.float32),
                "qscal": qscal,
                "blk3": blk3,
                "esc": esc_v.reshape(K, 1),
                "ebi": ebi_v.reshape(K, 1),
                "postc": postc_v.reshape(K, 1),
                "w1": w1_v,
                "w2": w2_v,
                "rest": np.ascontiguousarray(rest[b, r0 : r0 + RPC, :], np.float32),
            }
        )
    return in_maps


def kernel(pos, angle, node_type_edge, padding_mask, mask_aa, mask_pos, time_pos,
           means, stds, fp_w1, fp_w2, ang_w1, ang_w2, t_w1, t_b1, t_w2, t_b2):
    from concourse.bass_utils import run_bass_kernel_spmd

    key = "nc_v2"
    if key not in _COMPILED:
        _COMPILED[key] = _build_nc()
    nc = _COMPILED[key]

    in_maps = _prep_in_maps(
        pos, angle, padding_mask, mask_pos, time_pos, means, stds,
        fp_w1, fp_w2, ang_w1, ang_w2, t_w1, t_b1, t_w2, t_b2,
    )
    res = run_bass_kernel_spmd(nc, in_maps, core_ids=list(range(NCORES)))
    outs = [np.asarray(res.results[c]["out"], np.float32) for c in range(NCORES)]
    full = np.concatenate(outs, axis=0).reshape(B, N, E)
    return full


# revision 8
# speedup vs baseline: 1.0883x; 1.0342x over previous
"""Trainium2 Bass kernel for nn_Node3DEmbeddingv2 (gnn_message_passing).

Strategy (8 NeuronCores, SPMD, data-parallel over flattened (batch, query-row)):
  - 1536 query rows split into 8 x 192 (4 cores per batch).
  - Phase A (per 96-row strip): d^2 = |pi|^2 - 2 pi.pj + |pj|^2 via one fp16
    matmul against host-prepped 3-component fp16 splits of pos (24
    contraction rows, exact to f32), ACT Relu(x - 5e-4) to clamp the
    cancellation noise on the diagonal to exactly 0, ACT Sqrt, then an exact
    3-way fp16 split of d (33 mantissa bits).
  - Phase B (per 24-row block): flatten the fp16 d-components into a
    [3, 24*768] moving tile (partition trio 0-2), broadcast each row's 768
    distances across all 128 gaussian-channel partitions with a [3,128]-ones
    fp16 matmul per 512-col PSUM window (one stationary, reloaded cheaply /
    elided by ldw-opt). One ScalarE op per [128,1536] PSUM unit computes the
    Gaussian:
      Derivative_Erf(scale_k * d + bias_k) = 2/sqrt(pi) * exp(-((d-m_k)/s_k)^2/2)
    writing fp16. The key-axis sum runs as an in-place halving add-tree on
    DVE (fp16 tensor_tensor = 2x perf mode; tensor_reduce is 1x-capped) down
    to width 6, then one tiny f32 tensor_reduce into S.
  - Phase C: channel constants on the summed [128,192] tensor, fp16
    feature_proj MLP (gelu between two matmuls), DMA-transpose (fp16) back
    to row-major, add the host-computed angle/time tail, DMA out [192,512].
  - Host (numpy, negligible): angle MLP, sinusoidal time embedding MLP,
    masking, per-core input prep; all heavy compute is on-device.
"""

import math

import numpy as np

# Problem constants (hardcoded per the task contract).
B, N, K, E = 2, 768, 128, 512
INTER = E // 2
NCORES = 8
RPC = (B * N) // NCORES  # 192 rows per core
PI_REF = 3.14159         # matches reference's gaussian constant

NBLOCK = 8               # 24-row phase-B blocks per core
BR = 24                  # rows per block
NROWS_A = 24             # contraction rows of the d^2 matmul
D2_SHIFT = 5.0e-4        # relu clamp: zeroes |d| < 0.022 (true data min ~0.5)

# Derivative_Erf table semantics: d/dx erf(x) = 2/sqrt(pi) * exp(-x^2).
# DERF_INV converts the table output back to exp(-x^2).
DERF_INV = math.sqrt(math.pi) / 2.0

USE_LDW_OPT = False  # walrus rejects ldw-opt for these ldweights forms

_COMPILED = {}


def _enable_ldw_opt():
    """Flip walrus's redundant-LDWEIGHTS elimination on: the 288 broadcast
    matmuls reuse one stationary [3,128] ones matrix and the per-matmul
    reload serializes ~124ns each on the PE. Only safe with zero f32
    matmuls in the module (this kernel is all-fp16). Correctness is
    re-verified end-to-end against the oracle after any flag change."""
    from concourse import bass_utils

    if getattr(bass_utils, "_ldw_opt_patched", False):
        return
    orig_run = bass_utils.run_command

    def run_patched(argv, **kw):
        argv = [
            a.replace("--enable-ldw-opt=false", "--enable-ldw-opt=true")
            if isinstance(a, str) else a
            for a in argv
        ]
        return orig_run(argv, **kw)

    bass_utils.run_command = run_patched
    bass_utils._ldw_opt_patched = True


def _build_nc():
    import concourse.bass as bass
    import concourse.bacc as bacc
    from concourse import mybir
    from concourse.tile import TileContext

    if USE_LDW_OPT:
        _enable_ldw_opt()

    f32 = mybir.dt.float32
    f16 = mybir.dt.float16
    AF = mybir.ActivationFunctionType

    nc = bacc.Bacc("TRN2", target_bir_lowering=False)

    mkeys = nc.dram_tensor("mkeys", [NROWS_A, N], f16, kind="ExternalInput")
    squery = nc.dram_tensor("squery", [NROWS_A, RPC], f16, kind="ExternalInput")
    esc = nc.dram_tensor("esc", [K, 1], f32, kind="ExternalInput")
    ebi = nc.dram_tensor("ebi", [K, 1], f32, kind="ExternalInput")
    postc = nc.dram_tensor("postc", [K, 1], f32, kind="ExternalInput")
    w1 = nc.dram_tensor("w1", [K, K], f16, kind="ExternalInput")
    w2 = nc.dram_tensor("w2", [K, INTER], f16, kind="ExternalInput")
    rest = nc.dram_tensor("rest", [RPC, E], f32, kind="ExternalOutput" if False else "ExternalInput")
    out = nc.dram_tensor("out", [RPC, E], f32, kind="ExternalOutput")

    with TileContext(nc) as tc:
        with nc.allow_low_precision(reason="fp16 gaussian accumulate, verified vs oracle"), \
             tc.tile_pool(name="sb", bufs=1) as sb:
            # ---- constant loads ----
            mk_sb = sb.tile([NROWS_A, N], f16, tag="mk_sb")
            nc.sync.dma_start(out=mk_sb, in_=mkeys[:, :])
            sq_sb = sb.tile([NROWS_A, RPC], f16, tag="sq_sb")
            nc.sync.dma_start(out=sq_sb, in_=squery[:, :])
            esc_sb = sb.tile([K, 1], f32, tag="esc_sb")
            nc.sync.dma_start(out=esc_sb, in_=esc[:, :])
            ebi_sb = sb.tile([K, 1], f32, tag="ebi_sb")
            nc.sync.dma_start(out=ebi_sb, in_=ebi[:, :])
            postc_sb = sb.tile([K, 1], f32, tag="postc_sb")
            nc.sync.dma_start(out=postc_sb, in_=postc[:, :])
            w1_sb = sb.tile([K, K], f16, tag="w1_sb")
            nc.sync.dma_start(out=w1_sb, in_=w1[:, :])
            w2_sb = sb.tile([K, INTER], f16, tag="w2_sb")
            nc.sync.dma_start(out=w2_sb, in_=w2[:, :])
            ones3 = sb.tile([3, 128], f16, tag="ones3")
            nc.vector.memset(ones3, 1.0)
            shift_sb = sb.tile([96, 1], f32, tag="shift_sb")
            nc.vector.memset(shift_sb, -D2_SHIFT)

            S = sb.tile([K, RPC], f32, tag="S")

            # Collapse the input-DMA queue semaphores into one point.
            tc.strict_bb_all_engine_barrier()

            dcomps = {}

            def phase_a(st):
                # d^2 matmul + relu/sqrt + fp16 splits for one 96-row strip
                d2p = psA.tile([96, N], f32, tag="d2p", bufs=1)
                cols = slice(96 * st, 96 * (st + 1))
                nc.tensor.matmul(
                    d2p[:, 0:512], sq_sb[:, cols], mk_sb[:, 0:512],
                    start=True, stop=True,
                )
                nc.tensor.matmul(
                    d2p[:, 512:N], sq_sb[:, cols], mk_sb[:, 512:N],
                    start=True, stop=True,
                )
                dr = sb.tile([96, N], f32, tag="dr", bufs=2)
                nc.scalar.activation(dr, d2p, AF.Relu, bias=shift_sb)
                d32 = sb.tile([96, N], f32, tag="d32", bufs=2)
                nc.scalar.sqrt(d32, dr)
                dc = sb.tile([96, 3, N], f16, tag="dcomp", bufs=2)
                nc.vector.tensor_copy(dc[:, 0, :], d32)
                r1 = sb.tile([96, N], f32, tag="r1", bufs=2)
                nc.vector.tensor_sub(r1, d32, dc[:, 0, :])
                nc.vector.tensor_copy(dc[:, 1, :], r1)
                r2 = sb.tile([96, N], f32, tag="r2", bufs=2)
                nc.vector.tensor_sub(r2, r1, dc[:, 1, :])
                nc.vector.tensor_copy(dc[:, 2, :], r2)
                dcomps[st] = dc

            def bcast_matmul(out_ap, rhs, ldw):
                # nc.tensor.matmul with an explicit ldweights flag: all
                # broadcast matmuls share the ones3 stationary, so runs after
                # the first skip the per-matmul LDWEIGHTS reload (ldw=False).
                ifmap_ap = nc.tensor.lower_ap(rhs.opt({0}), opt=False)
                weights_ap = nc.tensor.lower_ap(
                    ones3.opt({0}), opt=False, for_matmul_weights=True
                )
                out_l = nc.tensor.lower_ap(out_ap)
                nc.tensor.add_instruction(
                    mybir.InstMatmult(
                        name=nc.get_next_instruction_name(),
                        replication_resolution=0,
                        replication_shift_amnt=0,
                        replication_num_rows=0,
                        start_tensor_calc=True,
                        stop_tensor_calc=True,
                        ins=[ifmap_ap, weights_ap],
                        outs=[out_l],
                        tile_position=(0, 0),
                        tile_size=(32, 128),
                        ldweights=ldw,
                    )
                )

            def phase_b(b):
                # broadcast + gaussian + key-sum for one 24-row block
                dc = dcomps[b // 4]
                p0 = BR * (b % 4)
                mfl = sb.tile([3, BR * N], f16, tag="mflat", bufs=2)
                for comp in range(3):
                    nc.gpsimd.dma_start(
                        out=mfl[comp : comp + 1, :].rearrange(
                            "p (i j) -> p i j", i=BR
                        ),
                        in_=dc[p0 : p0 + BR, comp, :],
                    )
                gsc = sb.tile([K, BR, N], f16, tag="gsc", bufs=2)
                for u in range(12):
                    pu = psB.tile([K, 1536], f32, tag="pu", bufs=2)
                    for w in range(3):
                        lo = 1536 * u + 512 * w
                        # reload weights on the first matmul after any
                        # other-stationary matmul ran on the PE (phase A of
                        # strip 1 is emitted between blocks 1 and 2)
                        ldw = u == 0 and w == 0 and b in (0, 2)
                        bcast_matmul(
                            pu[:, 512 * w : 512 * w + 512],
                            mfl[:, lo : lo + 512],
                            ldw,
                        )
                    nc.scalar.activation(
                        out=gsc[:, 2 * u : 2 * u + 2, :],
                        in_=pu,
                        func=AF.Derivative_Erf,
                        bias=ebi_sb,
                        scale=esc_sb,
                    )
                # in-place fp16 halving add-tree over the key axis
                wdt = N // 2
                while wdt >= 6:
                    nc.vector.tensor_add(
                        gsc[:, :, 0:wdt], gsc[:, :, 0:wdt],
                        gsc[:, :, wdt : 2 * wdt],
                    )
                    wdt //= 2
                nc.vector.tensor_reduce(
                    out=S[:, BR * b : BR * (b + 1)],
                    in_=gsc[:, :, 0:6],
                    axis=mybir.AxisListType.X,
                    op=mybir.AluOpType.add,
                )

            with tc.tile_pool(name="psA", bufs=1, space="PSUM") as psA, \
                 tc.tile_pool(name="psB", bufs=1, space="PSUM") as psB:
                phase_a(0)
                for b in range(NBLOCK):
                    if b == 2:
                        phase_a(1)
                    phase_b(b)

            # ---- phase C: channel constants + feature_proj MLP + output ----
            with tc.tile_pool(name="psC", bufs=1, space="PSUM") as psC:
                for st in range(2):
                    rows = slice(96 * st, 96 * (st + 1))
                    out_sb = sb.tile([96, E], f32, tag="out_sb", bufs=2)
                    nc.sync.dma_start(out=out_sb, in_=rest[rows, :])
                    nc.vector.tensor_scalar_mul(S[:, rows], S[:, rows], postc_sb)
                    s16 = sb.tile([K, 96], f16, tag="s16", bufs=2)
                    nc.vector.tensor_copy(s16, S[:, rows])
                    psum_h = psC.tile([K, 96], f32, tag="mlp_h", bufs=2)
                    nc.tensor.matmul(psum_h, w1_sb, s16, start=True, stop=True)
                    h16 = sb.tile([K, 96], f16, tag="h16", bufs=2)
                    nc.scalar.activation(h16, psum_h, AF.Gelu)
                    o16 = sb.tile([K, 2, 128], f16, tag="o16", bufs=2)
                    nc.vector.memset(o16, 0.0)
                    for e in range(2):
                        psum_o = psC.tile([K, 96], f32, tag="mlp_o", bufs=2)
                        nc.tensor.matmul(
                            psum_o, w2_sb[:, 128 * e : 128 * (e + 1)], h16,
                            start=True, stop=True,
                        )
                        nc.scalar.copy(o16[:, e, 0:96], psum_o)
                    tr16 = sb.tile([128, 2 * K], f16, tag="tr16", bufs=2)
                    for e in range(2):
                        nc.sync.dma_start_transpose(
                            tr16[:, 128 * e : 128 * (e + 1)], o16[:, e, :]
                        )
                    nc.vector.tensor_add(
                        out_sb[:, 0:INTER], out_sb[:, 0:INTER], tr16[0:96, :]
                    )
                    nc.sync.dma_start(out=out[rows, :], in_=out_sb)

    nc.compile()
    return nc


# ---------------- host-side prep (numpy) ----------------

def _erf_np(x):
    try:
        from scipy.special import erf
        return erf(x).astype(np.float32)
    except ImportError:
        f = np.frompyfunc(math.erf, 1, 1)
        return f(x.astype(np.float64)).astype(np.float32)


def _gelu_np(x):
    x = x.astype(np.float32)
    return (x * 0.5 * (1.0 + _erf_np(x / np.float32(math.sqrt(2.0))))).astype(
        np.float32
    )


def _silu_np(x):
    x = x.astype(np.float32)
    return (x / (1.0 + np.exp(-x))).astype(np.float32)


def _timestep_emb_np(t, dim):
    half = dim // 2
    freqs = np.exp(
        -np.log(10000.0) * np.arange(half, dtype=np.float32) / np.float32(half)
    ).astype(np.float32)
    a = t.astype(np.float32)[:, None] * freqs[None, :]
    return np.concatenate([np.sin(a), np.cos(a)], axis=-1).astype(np.float32)


def _host_tails(angle, mask_pos, time_pos, ang_w1, ang_w2, t_w1, t_b1, t_w2, t_b2):
    """rest[b, n, :] with rest[..., :INTER] = time_emb[..., :INTER] and
    rest[..., INTER:] = ang_f + time_emb[..., INTER:]."""
    angle = np.asarray(angle, np.float32)
    ang = np.where(np.isposinf(angle), np.float32(0.0), angle).astype(np.float32)
    ang_f = _gelu_np(ang @ np.asarray(ang_w1, np.float32)) @ np.asarray(
        ang_w2, np.float32
    )  # [B, N, INTER]

    def time_mlp(t):
        e = _timestep_emb_np(t, E)
        h = _silu_np(e @ np.asarray(t_w1, np.float32) + np.asarray(t_b1, np.float32))
        return (h @ np.asarray(t_w2, np.float32) + np.asarray(t_b2, np.float32)).astype(
            np.float32
        )

    tp = np.asarray(time_pos)
    te = time_mlp(tp)[:, None, :]                 # [B, 1, E]
    t0e = time_mlp(np.zeros_like(tp))[:, None, :]
    mask = np.asarray(mask_pos, bool)             # [B, N, 1]
    time_emb = np.where(mask, te, t0e).astype(np.float32)  # [B, N, E]

    rest = time_emb.copy()
    rest[..., INTER:] += ang_f.astype(np.float32)
    return rest.astype(np.float32)


def _split_f16(x, n):
    """n-component fp16 split: sum of components == x to n*11 mantissa bits."""
    comps = []
    r = np.asarray(x, np.float64)
    for _ in range(n):
        c = r.astype(np.float16).astype(np.float64)
        comps.append(c)
        r = r - c
    return comps


def _d2_gemm_operands(pos_b, pad_b, r0):
    """Host-prep the 24-row fp16 operands of the d^2 matmul.

    d^2(q, j) = |a|^2 - 2 a.b + |b|^2 with a = pos[q], b = pos[j]; every
    product of 3-component fp16 splits that matters to f32 precision gets
    its own contraction row (6 cross combos per coord + 3 |b|^2 comps
    against ones + 3 |a|^2 comps against ones)."""
    a = np.asarray(pos_b, np.float64)[r0 : r0 + RPC]     # queries [192, 3]
    bk = np.asarray(pos_b, np.float64).T.copy()          # keys    [3, N]
    if pad_b.any():
        bk[:, pad_b] = 1.0e6
    mk = np.zeros((NROWS_A, N), np.float16)
    sq = np.zeros((NROWS_A, RPC), np.float16)
    r = 0
    for c in range(3):
        bh, bm, bl = _split_f16(bk[c], 3)
        ah, am, al = _split_f16(a[:, c], 3)
        for (bc, ac) in ((bh, ah), (bm, ah), (bh, am),
                         (bl, ah), (bh, al), (bm, am)):
            mk[r] = bc.astype(np.float16)
            sq[r] = (-2.0 * ac).astype(np.float16)
            r += 1
    bsq = np.sum(bk * bk, axis=0)
    for comp in _split_f16(bsq, 3):
        mk[r] = comp.astype(np.float16)
        sq[r] = np.float16(1.0)
        r += 1
    asq = np.sum(a * a, axis=1)
    for comp in _split_f16(asq, 3):
        mk[r] = np.float16(1.0)
        sq[r] = comp.astype(np.float16)
        r += 1
    assert r == NROWS_A
    return mk, sq


def _prep_in_maps(pos, angle, padding_mask, mask_pos, time_pos,
                  means, stds, fp_w1, fp_w2, ang_w1, ang_w2,
                  t_w1, t_b1, t_w2, t_b2):
    pos = np.asarray(pos, np.float32)
    pad = np.asarray(padding_mask, bool)

    s = (np.abs(np.asarray(stds, np.float32)) + np.float32(0.01)).astype(np.float32)
    m = np.asarray(means, np.float32)
    inv_s = (np.float32(1.0) / s).astype(np.float32)
    # Derivative_Erf(x) with x = (d - m)/(s*sqrt(2))
    esc_v = (inv_s / np.float32(math.sqrt(2.0))).astype(np.float32)
    ebi_v = (-m * esc_v).astype(np.float32)
    postc_v = (
        np.float32(DERF_INV) / (np.float32(math.sqrt(2.0 * PI_REF)) * s)
    ).astype(np.float32)

    rest = _host_tails(
        angle, mask_pos, time_pos, ang_w1, ang_w2, t_w1, t_b1, t_w2, t_b2
    )

    w1_v = np.asarray(fp_w1, np.float16)
    w2_v = np.asarray(fp_w2, np.float16)

    in_maps = []
    for c in range(NCORES):
        b = c // (NCORES // B)
        r0 = (c % (NCORES // B)) * RPC
        mk, sq = _d2_gemm_operands(pos[b], pad[b], r0)
        in_maps.append(
            {
                "mkeys": mk,
                "squery": sq,
                "esc": esc_v.reshape(K, 1),
                "ebi": ebi_v.reshape(K, 1),
                "postc": postc_v.reshape(K, 1),
                "w1": w1_v,
                "w2": w2_v,
                "rest": np.ascontiguousarray(rest[b, r0 : r0 + RPC, :], np.float32),
            }
        )
    return in_maps


def kernel(pos, angle, node_type_edge, padding_mask, mask_aa, mask_pos, time_pos,
           means, stds, fp_w1, fp_w2, ang_w1, ang_w2, t_w1, t_b1, t_w2, t_b2):
    from concourse.bass_utils import run_bass_kernel_spmd

    key = "nc_v3"
    if key not in _COMPILED:
        _COMPILED[key] = _build_nc()
    nc = _COMPILED[key]

    in_maps = _prep_in_maps(
        pos, angle, padding_mask, mask_pos, time_pos, means, stds,
        fp_w1, fp_w2, ang_w1, ang_w2, t_w1, t_b1, t_w2, t_b2,
    )
    res = run_bass_kernel_spmd(nc, in_maps, core_ids=list(range(NCORES)))
    outs = [np.asarray(res.results[c]["out"], np.float32) for c in range(NCORES)]
    full = np.concatenate(outs, axis=0).reshape(B, N, E)
    return full
